# revision 2
# baseline (speedup 1.0000x reference)
"""Contrastive Predictive Coding loss kernel v2 for 8 Trainium2 NeuronCores.

Strategy (SPMD, batch row-sharded, per-core 1024 rows):
  - All activations transposed on chip ([D, batch], D on partitions); torch
    [in,out] weights used directly as lhsT. Host packs/folds weights.
  - Layer folds (host, f64): gi = h1s @ (we2@wih) + (be2@wih + bih + bhh)
    removes the encoder-l2 layer; q = h1p @ (wp2@we2^T) + bp2@we2^T removes
    the predictor-l2 layer (enc_b2's contribution is a per-row constant that
    cancels in LSE_i - logit_ii). GRU (1-z) computed as sigmoid(-x) directly.
  - fp8(e4m3) DoubleRow matmuls for the full-batch next_state layer-1 and the
    1024x8192 logits block; the local chain stays bf16.
  - Softmax stats on three engines: NS groups run fused exp+accum on ScalarE;
    the rest compute exp on VectorE via the bf16 Schraudolph bitcast
    (t = x*1846.64 + 16248.64 -> int16 -> bf16, bias-centered so
    E[approx/true] = 1), then reduce via two pairwise-add halvings on GpSimd
    (SBUF-only engine) and a final 512-wide reduce on VectorE.
  - diag(logits) via elementwise multiply + ones-vector matmul; result DMAed
    straight out of PSUM. log() of rowsums runs on host.
"""

import os
import sys

import numpy as np

for _p in ("/opt/trn_rl_repo", "/root/.axon_site/_ro/trn_rl_repo"):
    if os.path.isdir(_p) and _p not in sys.path:
        sys.path.append(_p)

D = 256
B = 8192
NCORES = 8
SH = B // NCORES          # 1024 rows per core
KP = D // 128             # 2 partition chunks of the contraction dim
NB = 512                  # matmul moving-operand block (one fp32 PSUM bank)
RT = SH // 128            # 8 row tiles of 128 pred rows
G = 2048                  # columns per exp/accum group (4 PSUM banks)
NG = B // G               # 4 groups per row tile
SCALE = 10.0              # 1 / temperature

EXP_A = SCALE * 128.0 / float(np.log(2.0))   # 1846.6407
EXP_B = 16256.0 - 7.364
NV = 12                   # of the 32 exp groups, how many go to V+Pool
NSPREAD = 28              # V-groups live in the first NSPREAD slots (clean tail)


def _is_v_group(idx):
    """Bresenham-spread NV V-groups among the first NSPREAD (g, t) slots."""
    if idx >= NSPREAD:
        return False
    return (idx * NV) // NSPREAD != ((idx + 1) * NV) // NSPREAD


_cache = {}


def _build():
    if "nc" in _cache:
        return _cache["nc"]

    import concourse.bacc as bacc
    import concourse.mybir as mybir
    import concourse.tile as tile

    dt = mybir.dt
    f32 = dt.float32
    bf16 = dt.bfloat16
    f8 = dt.float8e4
    i16 = dt.int16
    AF = mybir.ActivationFunctionType
    ALU = mybir.AluOpType
    AX = mybir.AxisListType
    DR = mybir.MatmulPerfMode.DoubleRow

    # packed bf16 weight columns inside wpk (per k-chunk of 128 rows)
    W_E1 = slice(0, D)                      # encoder l1
    W_G = slice(D, D + 3 * D)               # folded gru  (wgf = we2 @ wih)
    W_P1 = slice(4 * D, 5 * D)              # predictor l1
    W_Q = slice(5 * D, 6 * D)               # folded q    (wqf = wp2 @ we2^T)
    WPK_W = 6 * D

    nc = bacc.Bacc("TRN2", target_bir_lowering=False, num_devices=NCORES)

    xsT = nc.declare_dram_parameter("xsT", [D, SH], bf16, isOutput=False)
    xnTl = nc.declare_dram_parameter("xnTl", [D, SH], f8, isOutput=False)
    xnTf = nc.declare_dram_parameter("xnTf", [D, B], f8, isOutput=False)
    wpk_d = nc.declare_dram_parameter("wpk", [D, WPK_W], bf16, isOutput=False)
    w_e18 = nc.declare_dram_parameter("w_e18", [D, D], f8, isOutput=False)
    # bias columns: 0=enc_b1 2=gru_r 3=-(gru_z) 4=gru_n(bih) 5=gru_n(bhh)
    #               6=pred_b1 7=q (bp2 @ we2^T)
    bpk = nc.declare_dram_parameter("bpk", [D, 8], f32, isOutput=False)
    # out columns 0..RT-1 = exp-row-sums per row tile; out2 = raw diag dots
    out_d = nc.declare_dram_parameter("out", [128, 2 * RT], f32, isOutput=True)
    out2_d = nc.declare_dram_parameter("out2", [1, SH], f32, isOutput=True)

    with tile.TileContext(nc, num_cores=NCORES) as tc:
        with (
            tc.tile_pool(name="persist", bufs=1) as pp,
            tc.tile_pool(name="scratch", bufs=8) as sp,
        ):
            # ---- inputs, critical-path first: h1s needs xs + we1 (in wpk) ---
            # scalar queue: xs k=1, biases, wpk k=0, we18
            # sync queue:   xs k=0, wpk k=1, xnl8
            # gpsimd queue: xnf8 (SWDGE, chunked)
            we18 = pp.tile([128, KP, D], f8, name="we18")
            for k in range(KP):  # scalar queue first: layer-1 starts on these
                nc.scalar.dma_start(
                    out=we18[:, k, :], in_=w_e18[k * 128 : (k + 1) * 128, :]
                )
            bp = []
            for k in range(KP):
                t = pp.tile([128, 8], f32, name=f"bp_{k}")
                nc.scalar.dma_start(out=t[:, :], in_=bpk[k * 128 : (k + 1) * 128, :])
                bp.append(t)
            xs = []
            for k in range(KP):  # sync queue
                t = pp.tile([128, SH], bf16, name=f"xs_{k}")
                nc.sync.dma_start(out=t[:, :], in_=xsT[k * 128 : (k + 1) * 128, :])
                xs.append(t)
            wall = []
            for k in range(KP):
                t = pp.tile([128, WPK_W], bf16, name=f"wall_{k}")
                q_eng = nc.scalar if k == 0 else nc.sync
                q_eng.dma_start(out=t[:, :], in_=wpk_d[k * 128 : (k + 1) * 128, :])
                wall.append(t)
            xnl8 = pp.tile([128, KP, SH], f8, name="xnl8")
            for k in range(KP):  # sync queue (needed post-chain only)
                nc.sync.dma_start(
                    out=xnl8[:, k, :], in_=xnTl[k * 128 : (k + 1) * 128, :]
                )
            # full next_state (fp8): chunk 0 on gpsimd (needed first); the
            # bulk rides BEHIND the critical xs/wall loads on the serialized
            # sync/scalar queues so it can't steal their HBM bandwidth.
            xnf8 = pp.tile([128, KP, B], f8, name="xnf8")
            XCH = 2048
            for c0 in range(0, B, XCH):
                q_eng = (nc.gpsimd, nc.scalar, nc.sync, nc.sync)[c0 // XCH]
                for k in range(KP):
                    q_eng.dma_start(
                        out=xnf8[:, k, c0 : c0 + XCH],
                        in_=xnTf[k * 128 : (k + 1) * 128, c0 : c0 + XCH],
                    )
            ones = pp.tile([128, 1], bf16, name="ones")
            nc.vector.memset(ones[:, :], 1.0)

            def wslice(k, col, m):
                return wall[k][:, col.start + m * 128 : col.start + (m + 1) * 128]

            # persistent activations
            h1f8 = pp.tile([128, KP, B], f8, name="h1f8")
            qT8 = pp.tile([128, KP, SH], f8, name="qT8")
            dummy = pp.tile([128, G], f32, name="dummy")
            out_sb = pp.tile([128, 2 * RT], f32, name="out_sb")

            with tc.tile_pool(name="psx", bufs=2, space="PSUM") as psx:
                def ptile(name):
                    return psx.tile([128, G], f32, name=name, tag="px")

                def layer(dst, src, wcol, bias_col, act, tag):
                    """dst[m] (an AP) = act(w.T @ src + b) over SH cols; the
                    two 128-row output chunks epilogue on ScalarE (m=0) and
                    VectorE (m=1)."""
                    for m in range(KP):
                        ps = ptile(f"ps_{tag}_{m}")
                        for nb in range(SH // NB):
                            sl = slice(nb * NB, (nb + 1) * NB)
                            for k in range(KP):
                                nc.tensor.matmul(
                                    ps[:, sl],
                                    lhsT=wslice(k, wcol, m),
                                    rhs=src[k][:, sl],
                                    start=(k == 0),
                                    stop=(k == KP - 1),
                                )
                        bias = None if bias_col is None else bp[m][
                            :, bias_col : bias_col + 1
                        ]
                        if act == "relu" and m == 0:
                            nc.scalar.activation(
                                out=dst[m], in_=ps[:, :SH],
                                func=AF.Relu, bias=bias,
                            )
                        elif act == "relu":
                            nc.vector.tensor_scalar(
                                out=dst[m], in0=ps[:, :SH],
                                scalar1=bias, scalar2=0.0,
                                op0=ALU.add, op1=ALU.max,
                            )
                        elif act == "bias" and m == 0:
                            nc.scalar.activation(
                                out=dst[m], in_=ps[:, :SH],
                                func=AF.Identity, bias=bias,
                            )
                        else:
                            nc.vector.tensor_scalar(
                                out=dst[m], in0=ps[:, :SH],
                                scalar1=bias, scalar2=None, op0=ALU.add,
                            )

                def aps(tiles):
                    return [t[:, :] for t in tiles]

                # fp8 DoubleRow layer-1 over the FULL batch, one [128, G]
                # group at a time between chain stages.
                _l1_state = {"i": 0}

                def emit_l1(n=1):
                    for _ in range(n):
                        i = _l1_state["i"]
                        _l1_state["i"] += 1
                        if i >= KP * (B // G):
                            return
                        m, g0 = i % KP, (i // KP) * G
                        ps = ptile(f"ps_l1_{i}")
                        for s in range(G // NB):
                            sl = slice(s * NB, (s + 1) * NB)
                            gsl = slice(g0 + s * NB, g0 + (s + 1) * NB)
                            nc.tensor.matmul(
                                ps[:, sl],
                                lhsT=we18[:, :, m * 128 : (m + 1) * 128],
                                rhs=xnf8[:, :, gsl],
                                start=True, stop=True, perf_mode=DR,
                            )
                        dst = h1f8[:, m, g0 : g0 + G]
                        # alternate S/V; all run pre-logits so the exp act
                        # tables load once.
                        if i % 2 == 0:
                            nc.scalar.activation(
                                out=dst, in_=ps[:, :], func=AF.Relu,
                                bias=bp[m][:, 0:1],
                            )
                        else:
                            nc.vector.tensor_scalar(
                                out=dst, in0=ps[:, :],
                                scalar1=bp[m][:, 0:1], scalar2=0.0,
                                op0=ALU.add, op1=ALU.max,
                            )

                # layer-1 head start: PE crunches these while the chain's
                # weight DMAs land; continuous work also ramps the PE p-state.
                emit_l1(4)

                # ---- local chain: h1s -> GRU -> h1p -> q --------------------
                h1s = [sp.tile([128, SH], bf16, name=f"h1s_{k}", tag="scr") for k in range(KP)]
                layer(aps(h1s), xs, W_E1, 0, "relu", "h1s")
                emit_l1(2)

                def gate_psum(gate, m, tag):
                    ps = ptile(f"ps_{tag}_{m}")
                    col0 = W_G.start + gate * D + m * 128
                    for nb in range(SH // NB):
                        sl = slice(nb * NB, (nb + 1) * NB)
                        for k in range(KP):
                            nc.tensor.matmul(
                                ps[:, sl],
                                lhsT=wall[k][:, col0 : col0 + 128],
                                rhs=h1s[k][:, sl],
                                start=(k == 0),
                                stop=(k == KP - 1),
                            )
                    return ps

                rg = [sp.tile([128, SH], f32, name=f"rg_{m}", tag="scr") for m in range(KP)]
                zc = [sp.tile([128, SH], f32, name=f"zc_{m}", tag="scr") for m in range(KP)]
                ng = [sp.tile([128, SH], f32, name=f"ng_{m}", tag="scr") for m in range(KP)]
                ctx = [sp.tile([128, SH], bf16, name=f"ctx_{m}", tag="scr") for m in range(KP)]
                for m in range(KP):
                    ps_r = gate_psum(0, m, "gr")
                    nc.scalar.activation(
                        out=rg[m][:, :], in_=ps_r[:, :SH], func=AF.Sigmoid,
                        bias=bp[m][:, 2:3],
                    )
                emit_l1(2)
                for m in range(KP):
                    # zc = 1 - z = sigmoid(-(gi_z + b_z)); bias col3 is -b_z
                    ps_z = gate_psum(1, m, "gz")
                    nc.scalar.activation(
                        out=zc[m][:, :], in_=ps_z[:, :SH], func=AF.Sigmoid,
                        scale=-1.0, bias=bp[m][:, 3:4],
                    )
                for m in range(KP):
                    ps_n = gate_psum(2, m, "gn")
                    tmp = sp.tile([128, SH], f32, name=f"tmp_{m}", tag="scr")
                    # tmp = r * bhh_n + gi_n   (one fused DVE op)
                    nc.vector.scalar_tensor_tensor(
                        out=tmp[:, :], in0=rg[m][:, :], scalar=bp[m][:, 5:6],
                        in1=ps_n[:, :SH], op0=ALU.mult, op1=ALU.add,
                    )
                    nc.scalar.activation(
                        out=ng[m][:, :], in_=tmp[:, :], func=AF.Tanh,
                        bias=bp[m][:, 4:5],
                    )
                    nc.vector.tensor_tensor(
                        out=ctx[m][:, :], in0=zc[m][:, :], in1=ng[m][:, :],
                        op=ALU.mult,
                    )

                h1p = [sp.tile([128, SH], bf16, name=f"h1p_{k}", tag="scr") for k in range(KP)]
                layer(aps(h1p), ctx, W_P1, 6, "relu", "h1p")
                # q = h1p @ wqf + bq (identity epilogue, fp8 out)
                layer([qT8[:, 0, :], qT8[:, 1, :]], h1p, W_Q, 7, "bias", "q")

                # local next-state relu layer (for the diagonal): fp8 DR
                h1nl8 = pp.tile([128, KP, SH], f8, name="h1nl8")
                for m in range(KP):
                    ps = ptile(f"ps_h1nl_{m}")
                    for s in range(SH // NB):
                        sl = slice(s * NB, (s + 1) * NB)
                        nc.tensor.matmul(
                            ps[:, sl],
                            lhsT=we18[:, :, m * 128 : (m + 1) * 128],
                            rhs=xnl8[:, :, sl],
                            start=True, stop=True, perf_mode=DR,
                        )
                    nc.vector.tensor_scalar(
                        out=h1nl8[:, m, :], in0=ps[:, :SH],
                        scalar1=bp[m][:, 0:1], scalar2=0.0,
                        op0=ALU.add, op1=ALU.max,
                    )
                prod = [sp.tile([128, SH], bf16, name=f"prod_{k}", tag="scr") for k in range(KP)]
                for k in range(KP):
                    nc.vector.tensor_tensor(
                        out=prod[k][:, :], in0=qT8[:, k, :], in1=h1nl8[:, k, :],
                        op=ALU.mult,
                    )

                # ---- logits: fp8 DR matmuls; exp on S or V+Pool -------------
                sums = pp.tile([128, RT * NG], f32, name="sums")
                for g in range(NG):
                    for t in range(RT):
                        pl = ptile("pl")
                        for s in range(G // NB):
                            c0 = g * G + s * NB
                            sl = slice(s * NB, (s + 1) * NB)
                            nc.tensor.matmul(
                                pl[:, sl],
                                lhsT=qT8[:, :, t * 128 : (t + 1) * 128],
                                rhs=h1f8[:, :, c0 : c0 + NB],
                                start=True, stop=True, perf_mode=DR,
                            )
                        col = t * NG + g
                        if _is_v_group(g * RT + t):
                            ex = sp.tile([128, G], i16, name="ex", tag="ex")
                            nc.vector.tensor_scalar(
                                out=ex[:, :], in0=pl[:, :],
                                scalar1=EXP_A, scalar2=EXP_B,
                                op0=ALU.mult, op1=ALU.add,
                            )
                            ca = sp.tile([128, G // 2], bf16, name="ca", tag="ca")
                            nc.gpsimd.tensor_tensor(
                                out=ca[:, :],
                                in0=ex[:, 0 : G // 2].bitcast(bf16),
                                in1=ex[:, G // 2 : G].bitcast(bf16),
                                op=ALU.add,
                            )
                            cb = sp.tile([128, G // 4], bf16, name="cb", tag="cb")
                            nc.gpsimd.tensor_tensor(
                                out=cb[:, :],
                                in0=ca[:, 0 : G // 4],
                                in1=ca[:, G // 4 : G // 2],
                                op=ALU.add,
                            )
                            nc.vector.tensor_reduce(
                                out=sums[:, col : col + 1],
                                in_=cb[:, :], axis=AX.X, op=ALU.add,
                            )
                        else:
                            nc.scalar.activation(
                                out=dummy[:, :], in_=pl[:, :], func=AF.Exp,
                                scale=SCALE,
                                accum_out=sums[:, col : col + 1],
                            )
                # ---- diag: rowsum(q8 * h1nl8) via ones-vector matmul --------
                dps = ptile("dps")
                for s in range(SH // NB):
                    sl = slice(s * NB, (s + 1) * NB)
                    for k in range(KP):
                        nc.tensor.matmul(
                            dps[:1, sl],
                            lhsT=ones[:, :],
                            rhs=prod[k][:, sl],
                            start=(k == 0),
                            stop=(k == KP - 1),
                        )
                d2 = pp.tile([1, SH], f32, name="d2")
                nc.scalar.copy(d2[:, :], dps[:1, :SH])
                nc.sync.dma_start(out=out2_d[:, :], in_=d2[:, :])
                for t in range(RT):
                    nc.vector.reduce_sum(
                        out=out_sb[:, t : t + 1],
                        in_=sums[:, t * NG : (t + 1) * NG], axis=AX.X,
                    )

            nc.sync.dma_start(out=out_d[:, :], in_=out_sb[:, :])

    if not nc.is_finalized():
        nc.finalize()
    _cache["nc"] = nc
    return nc


def _prep_in_maps(inputs):
    import ml_dtypes

    bf = ml_dtypes.bfloat16
    f8 = ml_dtypes.float8_e4m3
    f = lambda x: np.asarray(x).astype(np.float64)
    state = f(inputs["state"])
    next_state = f(inputs["next_state"])
    we1 = f(inputs["enc_w1"])
    we2 = f(inputs["enc_w2"])
    wih = f(inputs["gru_wih"])
    wp1 = f(inputs["pred_w1"])
    wp2 = f(inputs["pred_w2"])
    be1 = f(inputs["enc_b1"])
    be2 = f(inputs["enc_b2"])
    bih = f(inputs["gru_bih"])
    bhh = f(inputs["gru_bhh"])
    bp1 = f(inputs["pred_b1"])
    bp2 = f(inputs["pred_b2"])

    wgf = we2 @ wih                    # [D, 3D] folded gru weights
    bgf = be2 @ wih + bih              # [3D]    folded gi bias (+bih)
    wqf = wp2 @ we2.T                  # [D, D]  folded q weights
    bqf = bp2 @ we2.T                  # [D]     folded q bias

    bias_pack = np.stack(
        [
            be1,
            np.zeros(D),
            bgf[:D] + bhh[:D],                    # r-gate bias
            -(bgf[D : 2 * D] + bhh[D : 2 * D]),   # NEGATED z-gate bias
            bgf[2 * D :],                         # n-gate (bih part)
            bhh[2 * D :],                         # n-gate (r * bhh part)
            bp1,
            bqf,
        ],
        axis=1,
    ).astype(np.float32)

    wpk = np.concatenate([we1, wgf, wp1, wqf], axis=1)  # [D, 6D]
    nT8 = np.ascontiguousarray(next_state.T.astype(np.float32)).astype(f8)
    sT = np.ascontiguousarray(state.T.astype(np.float32)).astype(bf)
    shared = {
        "wpk": np.ascontiguousarray(wpk.astype(np.float32)).astype(bf),
        "w_e18": we1.astype(np.float32).astype(f8),
        "bpk": np.ascontiguousarray(bias_pack),
        "xnTf": nT8,
    }
    in_maps = []
    for c in range(NCORES):
        sl = slice(c * SH, (c + 1) * SH)
        in_maps.append(
            {
                "xsT": np.ascontiguousarray(sT[:, sl]),
                "xnTl": np.ascontiguousarray(nT8[:, sl]),
                **shared,
            }
        )
    return in_maps


last_results = None


def _finish(results):
    total = 0.0
    for r in results:
        rowsum = r["out"].astype(np.float64)[:, :RT]  # [128, RT], col t = tile
        diag = r["out2"].astype(np.float64).reshape(-1)  # [SH] raw dots
        lse_flat = np.log(rowsum).T.reshape(-1)  # row j = t*128+p
        total += float((lse_flat - SCALE * diag).sum())
    return np.float32(total / B)


def kernel(**inputs) -> np.ndarray:
    from concourse.bass_utils import run_bass_kernel_spmd

    global last_results
    nc = _build()
    in_maps = _prep_in_maps(inputs)
    res = run_bass_kernel_spmd(nc, in_maps, core_ids=list(range(NCORES)))
    last_results = res
    return _finish(res.results)


# ---------------------------------------------------------------------------
# Pure-numpy golden model of the exact device algorithm (for test.py).
def golden(**inputs) -> np.ndarray:
    import ml_dtypes

    in_maps = _prep_in_maps(inputs)
    f32 = np.float32
    m0 = in_maps[0]
    bfd = ml_dtypes.bfloat16
    f8 = ml_dtypes.float8_e4m3

    def as32(x):
        return np.asarray(x).astype(f32)

    wpk = as32(m0["wpk"])
    we1 = wpk[:, :D]
    wgf = wpk[:, D : 4 * D]
    wp1 = wpk[:, 4 * D : 5 * D]
    wqf = wpk[:, 5 * D : 6 * D]
    bpk = m0["bpk"]
    we18 = as32(m0["w_e18"])

    # full fp8 layer-1 (same on every core)
    h1f = np.maximum(as32(m0["xnTf"]).T @ we18 + bpk[:, 0], 0.0)
    h1f8 = h1f.astype(f8).astype(f32)  # [B, D]

    def schraud(x):  # x = raw dot (unscaled); device V-group exp approx
        t = np.rint(x * EXP_A + EXP_B).astype(np.int16)
        return t.view(bfd).astype(f32)

    def b16(x):
        return x.astype(bfd).astype(f32)

    results = []
    for c in range(NCORES):
        m = in_maps[c]
        xsv = as32(m["xsT"]).T
        h1 = b16(np.maximum(xsv @ we1 + bpk[:, 0], 0.0))
        gi = h1 @ wgf
        r = 1.0 / (1.0 + np.exp(-(gi[:, :D] + bpk[:, 2])))
        zc = 1.0 / (1.0 + np.exp(gi[:, D : 2 * D] - bpk[:, 3]))
        n = np.tanh(gi[:, 2 * D :] + bpk[:, 4] + r * bpk[:, 5])
        ctx = b16(zc * n)
        h1p = b16(np.maximum(ctx @ wp1 + bpk[:, 6], 0.0))
        q8 = (h1p @ wqf + bpk[:, 7]).astype(f8).astype(f32)
        dots = q8 @ h1f8.T  # [SH, B] raw (unscaled)
        rowsum = np.zeros((SH,), f32)
        for g in range(NG):
            for t in range(RT):
                sub = dots[t * 128 : (t + 1) * 128, g * G : (g + 1) * G]
                if _is_v_group(g * RT + t):
                    es = schraud(sub)
                    ca = b16(es[:, : G // 2] + es[:, G // 2 :])
                    cb = b16(ca[:, : G // 4] + ca[:, G // 4 : G // 2])
                    s = cb.sum(axis=1)
                else:
                    s = np.exp(SCALE * sub.astype(np.float64)).astype(
                        np.float64
                    ).sum(axis=1)
                rowsum[t * 128 : (t + 1) * 128] += s.astype(f32)
        h1l = np.maximum(as32(m["xnTl"]).T @ we18 + bpk[:, 0], 0.0)
        h1l8 = h1l.astype(f8).astype(f32)
        prod = b16(q8 * h1l8)
        diag = prod.sum(axis=1)
        out = np.concatenate(
            [rowsum.reshape(RT, 128).T, np.zeros((128, RT), f32)], axis=1
        )
        results.append({"out": out, "out2": diag.reshape(1, SH)})
    return _finish(results)


# revision 3
# speedup vs baseline: 7685.5428x; 7685.5428x over previous
"""Contrastive Predictive Coding loss kernel v2 for 8 Trainium2 NeuronCores.

Strategy (SPMD, batch row-sharded, per-core 1024 rows):
  - All activations transposed on chip ([D, batch], D on partitions); torch
    [in,out] weights used directly as lhsT. Host packs/folds weights.
  - Layer folds (host, f64): gi = h1s @ (we2@wih) + (be2@wih + bih + bhh)
    removes the encoder-l2 layer; q = h1p @ (wp2@we2^T) + bp2@we2^T removes
    the predictor-l2 layer (enc_b2's contribution is a per-row constant that
    cancels in LSE_i - logit_ii). GRU (1-z) computed as sigmoid(-x) directly.
  - fp8(e4m3) DoubleRow matmuls for the full-batch next_state layer-1 and the
    1024x8192 logits block; the local chain stays bf16.
  - Softmax stats on three engines: NS groups run fused exp+accum on ScalarE;
    the rest compute exp on VectorE via the bf16 Schraudolph bitcast
    (t = x*1846.64 + 16248.64 -> int16 -> bf16, bias-centered so
    E[approx/true] = 1), then reduce via two pairwise-add halvings on GpSimd
    (SBUF-only engine) and a final 512-wide reduce on VectorE.
  - diag(logits) via elementwise multiply + ones-vector matmul; result DMAed
    straight out of PSUM. log() of rowsums runs on host.
"""

import os
import sys

import numpy as np

for _p in ("/opt/trn_rl_repo", "/root/.axon_site/_ro/trn_rl_repo"):
    if os.path.isdir(_p) and _p not in sys.path:
        sys.path.append(_p)

D = 256
B = 8192
NCORES = 8
SH = B // NCORES          # 1024 rows per core
KP = D // 128             # 2 partition chunks of the contraction dim
NB = 512                  # matmul moving-operand block (one fp32 PSUM bank)
RT = SH // 128            # 8 row tiles of 128 pred rows
G = 2048                  # columns per exp/accum group (4 PSUM banks)
NG = B // G               # 4 groups per row tile
SCALE = 10.0              # 1 / temperature

EXP_A = SCALE * 128.0 / float(np.log(2.0))   # 1846.6407
EXP_B = 16256.0 - 7.364
NV = 12                   # of the 32 exp groups, how many go to V+Pool
NSPREAD = 28              # V-groups live in the first NSPREAD slots (clean tail)


def _is_v_group(idx):
    """Bresenham-spread NV V-groups among the first NSPREAD (g, t) slots."""
    if idx >= NSPREAD:
        return False
    return (idx * NV) // NSPREAD != ((idx + 1) * NV) // NSPREAD


_cache = {}


def _build():
    if "nc" in _cache:
        return _cache["nc"]

    import concourse.bacc as bacc
    import concourse.mybir as mybir
    import concourse.tile as tile

    dt = mybir.dt
    f32 = dt.float32
    bf16 = dt.bfloat16
    f8 = dt.float8e4
    i16 = dt.int16
    AF = mybir.ActivationFunctionType
    ALU = mybir.AluOpType
    AX = mybir.AxisListType
    DR = mybir.MatmulPerfMode.DoubleRow

    # packed bf16 weight columns inside wpk (per k-chunk of 128 rows)
    W_G = slice(0, 3 * D)                   # folded gru  (wgf = we2 @ wih)
    W_P1 = slice(3 * D, 4 * D)              # predictor l1
    W_Q = slice(4 * D, 5 * D)               # folded q    (wqf = wp2 @ we2^T)
    WPK_W = 5 * D

    nc = bacc.Bacc("TRN2", target_bir_lowering=False, num_devices=NCORES)

    xsT = nc.declare_dram_parameter("xsT", [D, SH], bf16, isOutput=False)
    xnTl = nc.declare_dram_parameter("xnTl", [D, SH], f8, isOutput=False)
    xnTf = nc.declare_dram_parameter("xnTf", [D, B], f8, isOutput=False)
    wpk_d = nc.declare_dram_parameter("wpk", [D, WPK_W], bf16, isOutput=False)
    we1_d = nc.declare_dram_parameter("we1b", [D, D], bf16, isOutput=False)
    w_e18 = nc.declare_dram_parameter("w_e18", [D, D], f8, isOutput=False)
    # bias columns: 0=enc_b1 2=gru_r 3=-(gru_z) 4=gru_n(bih) 5=gru_n(bhh)
    #               6=pred_b1 7=q (bp2 @ we2^T)
    bpk = nc.declare_dram_parameter("bpk", [D, 8], f32, isOutput=False)
    # out columns 0..RT-1 = exp-row-sums per row tile; out2 = raw diag dots
    out_d = nc.declare_dram_parameter("out", [128, 2 * RT], f32, isOutput=True)
    out2_d = nc.declare_dram_parameter("out2", [1, SH], f32, isOutput=True)

    with tile.TileContext(nc, num_cores=NCORES) as tc:
        with (
            tc.tile_pool(name="persist", bufs=1) as pp,
            tc.tile_pool(name="scratch", bufs=8) as sp,
        ):
            # ---- inputs, critical-path first: h1s needs xs + we1 (in wpk) ---
            # scalar queue: xs k=1, biases, wpk k=0, we18
            # sync queue:   xs k=0, wpk k=1, xnl8
            # gpsimd queue: xnf8 (SWDGE, chunked)
            we18 = pp.tile([128, KP, D], f8, name="we18")
            for k in range(KP):  # scalar queue first: layer-1 starts on these
                nc.scalar.dma_start(
                    out=we18[:, k, :], in_=w_e18[k * 128 : (k + 1) * 128, :]
                )
            we1t = []
            for k in range(KP):
                t = pp.tile([128, D], bf16, name=f"we1_{k}")
                q_eng = nc.scalar if k == 0 else nc.sync
                q_eng.dma_start(out=t[:, :], in_=we1_d[k * 128 : (k + 1) * 128, :])
                we1t.append(t)
            bp = []
            for k in range(KP):
                t = pp.tile([128, 8], f32, name=f"bp_{k}")
                nc.scalar.dma_start(out=t[:, :], in_=bpk[k * 128 : (k + 1) * 128, :])
                bp.append(t)
            xs = []
            for k in range(KP):  # sync queue
                t = pp.tile([128, SH], bf16, name=f"xs_{k}")
                nc.sync.dma_start(out=t[:, :], in_=xsT[k * 128 : (k + 1) * 128, :])
                xs.append(t)
            wall = []
            for k in range(KP):
                t = pp.tile([128, WPK_W], bf16, name=f"wall_{k}")
                q_eng = nc.scalar if k == 0 else nc.sync
                q_eng.dma_start(out=t[:, :], in_=wpk_d[k * 128 : (k + 1) * 128, :])
                wall.append(t)
            xnl8 = pp.tile([128, KP, SH], f8, name="xnl8")
            for k in range(KP):  # sync queue (needed post-chain only)
                nc.sync.dma_start(
                    out=xnl8[:, k, :], in_=xnTl[k * 128 : (k + 1) * 128, :]
                )
            # full next_state (fp8): chunk 0 on gpsimd (needed first); the
            # bulk rides BEHIND the critical xs/wall loads on the serialized
            # sync/scalar queues so it can't steal their HBM bandwidth.
            xnf8 = pp.tile([128, KP, B], f8, name="xnf8")
            XCH = 2048
            for c0 in range(0, B, XCH):
                q_eng = (nc.gpsimd, nc.scalar, nc.sync, nc.sync)[c0 // XCH]
                for k in range(KP):
                    q_eng.dma_start(
                        out=xnf8[:, k, c0 : c0 + XCH],
                        in_=xnTf[k * 128 : (k + 1) * 128, c0 : c0 + XCH],
                    )
            ones = pp.tile([128, 1], bf16, name="ones")
            nc.vector.memset(ones[:, :], 1.0)

            def wslice(k, col, m):
                return wall[k][:, col.start + m * 128 : col.start + (m + 1) * 128]

            # persistent activations
            h1f8 = pp.tile([128, KP, B], f8, name="h1f8")
            qT8 = pp.tile([128, KP, SH], f8, name="qT8")
            dummy = pp.tile([128, G], f32, name="dummy")
            out_sb = pp.tile([128, 2 * RT], f32, name="out_sb")

            with tc.tile_pool(name="psx", bufs=2, space="PSUM") as psx:
                def ptile(name):
                    return psx.tile([128, G], f32, name=name, tag="px")

                def layer(dst, src, wcol, bias_col, act, tag, wtiles=None):
                    """dst[m] (an AP) = act(w.T @ src + b) over SH cols; the
                    two 128-row output chunks epilogue on ScalarE (m=0) and
                    VectorE (m=1)."""
                    for m in range(KP):
                        ps = ptile(f"ps_{tag}_{m}")
                        for nb in range(SH // NB):
                            sl = slice(nb * NB, (nb + 1) * NB)
                            for k in range(KP):
                                lhsT = (wtiles[k][:, m * 128 : (m + 1) * 128]
                                        if wtiles is not None
                                        else wslice(k, wcol, m))
                                nc.tensor.matmul(
                                    ps[:, sl],
                                    lhsT=lhsT,
                                    rhs=src[k][:, sl],
                                    start=(k == 0),
                                    stop=(k == KP - 1),
                                )
                        bias = None if bias_col is None else bp[m][
                            :, bias_col : bias_col + 1
                        ]
                        if act == "relu" and m == 0:
                            nc.scalar.activation(
                                out=dst[m], in_=ps[:, :SH],
                                func=AF.Relu, bias=bias,
                            )
                        elif act == "relu":
                            nc.vector.tensor_scalar(
                                out=dst[m], in0=ps[:, :SH],
                                scalar1=bias, scalar2=0.0,
                                op0=ALU.add, op1=ALU.max,
                            )
                        elif act == "bias" and m == 0:
                            nc.scalar.activation(
                                out=dst[m], in_=ps[:, :SH],
                                func=AF.Identity, bias=bias,
                            )
                        else:
                            nc.vector.tensor_scalar(
                                out=dst[m], in0=ps[:, :SH],
                                scalar1=bias, scalar2=None, op0=ALU.add,
                            )

                def aps(tiles):
                    return [t[:, :] for t in tiles]

                # fp8 DoubleRow layer-1 over the FULL batch, one [128, G]
                # group at a time between chain stages.
                _l1_state = {"i": 0}

                def emit_l1(n=1):
                    for _ in range(n):
                        i = _l1_state["i"]
                        _l1_state["i"] += 1
                        if i >= KP * (B // G):
                            return
                        m, g0 = i % KP, (i // KP) * G
                        ps = ptile(f"ps_l1_{i}")
                        for s in range(G // NB):
                            sl = slice(s * NB, (s + 1) * NB)
                            gsl = slice(g0 + s * NB, g0 + (s + 1) * NB)
                            nc.tensor.matmul(
                                ps[:, sl],
                                lhsT=we18[:, :, m * 128 : (m + 1) * 128],
                                rhs=xnf8[:, :, gsl],
                                start=True, stop=True, perf_mode=DR,
                            )
                        dst = h1f8[:, m, g0 : g0 + G]
                        # alternate S/V; all run pre-logits so the exp act
                        # tables load once.
                        if i % 2 == 0:
                            nc.scalar.activation(
                                out=dst, in_=ps[:, :], func=AF.Relu,
                                bias=bp[m][:, 0:1],
                            )
                        else:
                            nc.vector.tensor_scalar(
                                out=dst, in0=ps[:, :],
                                scalar1=bp[m][:, 0:1], scalar2=0.0,
                                op0=ALU.add, op1=ALU.max,
                            )

                # layer-1 head start: PE crunches these while the chain's
                # weight DMAs land; continuous work also ramps the PE p-state.
                emit_l1(4)

                # ---- local chain: h1s -> GRU -> h1p -> q --------------------
                h1s = [sp.tile([128, SH], bf16, name=f"h1s_{k}", tag="scr") for k in range(KP)]
                layer(aps(h1s), xs, None, 0, "relu", "h1s", wtiles=we1t)
                emit_l1(2)

                def gate_psum(gate, m, tag):
                    ps = ptile(f"ps_{tag}_{m}")
                    col0 = W_G.start + gate * D + m * 128
                    for nb in range(SH // NB):
                        sl = slice(nb * NB, (nb + 1) * NB)
                        for k in range(KP):
                            nc.tensor.matmul(
                                ps[:, sl],
                                lhsT=wall[k][:, col0 : col0 + 128],
                                rhs=h1s[k][:, sl],
                                start=(k == 0),
                                stop=(k == KP - 1),
                            )
                    return ps

                rg = [sp.tile([128, SH], f32, name=f"rg_{m}", tag="scr") for m in range(KP)]
                zc = [sp.tile([128, SH], f32, name=f"zc_{m}", tag="scr") for m in range(KP)]
                ng = [sp.tile([128, SH], f32, name=f"ng_{m}", tag="scr") for m in range(KP)]
                ctx = [sp.tile([128, SH], bf16, name=f"ctx_{m}", tag="scr") for m in range(KP)]
                for m in range(KP):
                    ps_r = gate_psum(0, m, "gr")
                    nc.scalar.activation(
                        out=rg[m][:, :], in_=ps_r[:, :SH], func=AF.Sigmoid,
                        bias=bp[m][:, 2:3],
                    )
                emit_l1(2)
                for m in range(KP):
                    # zc = 1 - z = sigmoid(-(gi_z + b_z)); bias col3 is -b_z
                    ps_z = gate_psum(1, m, "gz")
                    nc.scalar.activation(
                        out=zc[m][:, :], in_=ps_z[:, :SH], func=AF.Sigmoid,
                        scale=-1.0, bias=bp[m][:, 3:4],
                    )
                for m in range(KP):
                    ps_n = gate_psum(2, m, "gn")
                    tmp = sp.tile([128, SH], f32, name=f"tmp_{m}", tag="scr")
                    # tmp = r * bhh_n + gi_n   (one fused DVE op)
                    nc.vector.scalar_tensor_tensor(
                        out=tmp[:, :], in0=rg[m][:, :], scalar=bp[m][:, 5:6],
                        in1=ps_n[:, :SH], op0=ALU.mult, op1=ALU.add,
                    )
                    nc.scalar.activation(
                        out=ng[m][:, :], in_=tmp[:, :], func=AF.Tanh,
                        bias=bp[m][:, 4:5],
                    )
                    nc.vector.tensor_tensor(
                        out=ctx[m][:, :], in0=zc[m][:, :], in1=ng[m][:, :],
                        op=ALU.mult,
                    )

                h1p = [sp.tile([128, SH], bf16, name=f"h1p_{k}", tag="scr") for k in range(KP)]
                layer(aps(h1p), ctx, W_P1, 6, "relu", "h1p")
                # q = h1p @ wqf + bq (identity epilogue, fp8 out)
                layer([qT8[:, 0, :], qT8[:, 1, :]], h1p, W_Q, 7, "bias", "q")

                # local next-state relu layer (for the diagonal): fp8 DR
                h1nl8 = pp.tile([128, KP, SH], f8, name="h1nl8")
                for m in range(KP):
                    ps = ptile(f"ps_h1nl_{m}")
                    for s in range(SH // NB):
                        sl = slice(s * NB, (s + 1) * NB)
                        nc.tensor.matmul(
                            ps[:, sl],
                            lhsT=we18[:, :, m * 128 : (m + 1) * 128],
                            rhs=xnl8[:, :, sl],
                            start=True, stop=True, perf_mode=DR,
                        )
                    nc.vector.tensor_scalar(
                        out=h1nl8[:, m, :], in0=ps[:, :SH],
                        scalar1=bp[m][:, 0:1], scalar2=0.0,
                        op0=ALU.add, op1=ALU.max,
                    )
                prod = [sp.tile([128, SH], bf16, name=f"prod_{k}", tag="scr") for k in range(KP)]
                for k in range(KP):
                    nc.vector.tensor_tensor(
                        out=prod[k][:, :], in0=qT8[:, k, :], in1=h1nl8[:, k, :],
                        op=ALU.mult,
                    )
                # ---- diag: rowsum(q8 * h1nl8) via ones-vector matmul, done
                # up front so the tail is just the final reduces + DMA.
                dps = ptile("dps")
                for s in range(SH // NB):
                    sl = slice(s * NB, (s + 1) * NB)
                    for k in range(KP):
                        nc.tensor.matmul(
                            dps[:1, sl],
                            lhsT=ones[:, :],
                            rhs=prod[k][:, sl],
                            start=(k == 0),
                            stop=(k == KP - 1),
                        )
                d2 = pp.tile([1, SH], f32, name="d2")
                nc.scalar.copy(d2[:, :], dps[:1, :SH])
                nc.sync.dma_start(out=out2_d[:, :], in_=d2[:, :])

                # ---- logits: fp8 DR matmuls; exp on S or V+Pool -------------
                sums = pp.tile([128, RT * NG], f32, name="sums")
                for g in range(NG):
                    for t in range(RT):
                        pl = ptile("pl")
                        for s in range(G // NB):
                            c0 = g * G + s * NB
                            sl = slice(s * NB, (s + 1) * NB)
                            nc.tensor.matmul(
                                pl[:, sl],
                                lhsT=qT8[:, :, t * 128 : (t + 1) * 128],
                                rhs=h1f8[:, :, c0 : c0 + NB],
                                start=True, stop=True, perf_mode=DR,
                            )
                        col = t * NG + g
                        if _is_v_group(g * RT + t):
                            ex = sp.tile([128, G], i16, name="ex", tag="ex")
                            nc.vector.tensor_scalar(
                                out=ex[:, :], in0=pl[:, :],
                                scalar1=EXP_A, scalar2=EXP_B,
                                op0=ALU.mult, op1=ALU.add,
                            )
                            ca = sp.tile([128, G // 2], bf16, name="ca", tag="ca")
                            nc.gpsimd.tensor_tensor(
                                out=ca[:, :],
                                in0=ex[:, 0 : G // 2].bitcast(bf16),
                                in1=ex[:, G // 2 : G].bitcast(bf16),
                                op=ALU.add,
                            )
                            cb = sp.tile([128, G // 4], bf16, name="cb", tag="cb")
                            nc.gpsimd.tensor_tensor(
                                out=cb[:, :],
                                in0=ca[:, 0 : G // 4],
                                in1=ca[:, G // 4 : G // 2],
                                op=ALU.add,
                            )
                            nc.vector.tensor_reduce(
                                out=sums[:, col : col + 1],
                                in_=cb[:, :], axis=AX.X, op=ALU.add,
                            )
                        else:
                            nc.scalar.activation(
                                out=dummy[:, :], in_=pl[:, :], func=AF.Exp,
                                scale=SCALE,
                                accum_out=sums[:, col : col + 1],
                            )
                for t in range(RT):
                    nc.vector.reduce_sum(
                        out=out_sb[:, t : t + 1],
                        in_=sums[:, t * NG : (t + 1) * NG], axis=AX.X,
                    )

            nc.sync.dma_start(out=out_d[:, :], in_=out_sb[:, :])

    if not nc.is_finalized():
        nc.finalize()
    _cache["nc"] = nc
    return nc


def _prep_in_maps(inputs):
    import ml_dtypes

    bf = ml_dtypes.bfloat16
    f8 = ml_dtypes.float8_e4m3
    f = lambda x: np.asarray(x).astype(np.float64)
    state = f(inputs["state"])
    next_state = f(inputs["next_state"])
    we1 = f(inputs["enc_w1"])
    we2 = f(inputs["enc_w2"])
    wih = f(inputs["gru_wih"])
    wp1 = f(inputs["pred_w1"])
    wp2 = f(inputs["pred_w2"])
    be1 = f(inputs["enc_b1"])
    be2 = f(inputs["enc_b2"])
    bih = f(inputs["gru_bih"])
    bhh = f(inputs["gru_bhh"])
    bp1 = f(inputs["pred_b1"])
    bp2 = f(inputs["pred_b2"])

    wgf = we2 @ wih                    # [D, 3D] folded gru weights
    bgf = be2 @ wih + bih              # [3D]    folded gi bias (+bih)
    wqf = wp2 @ we2.T                  # [D, D]  folded q weights
    bqf = bp2 @ we2.T                  # [D]     folded q bias

    bias_pack = np.stack(
        [
            be1,
            np.zeros(D),
            bgf[:D] + bhh[:D],                    # r-gate bias
            -(bgf[D : 2 * D] + bhh[D : 2 * D]),   # NEGATED z-gate bias
            bgf[2 * D :],                         # n-gate (bih part)
            bhh[2 * D :],                         # n-gate (r * bhh part)
            bp1,
            bqf,
        ],
        axis=1,
    ).astype(np.float32)

    wpk = np.concatenate([wgf, wp1, wqf], axis=1)  # [D, 5D]
    nT8 = np.ascontiguousarray(next_state.T.astype(np.float32)).astype(f8)
    sT = np.ascontiguousarray(state.T.astype(np.float32)).astype(bf)
    shared = {
        "wpk": np.ascontiguousarray(wpk.astype(np.float32)).astype(bf),
        "we1b": np.ascontiguousarray(we1.astype(np.float32)).astype(bf),
        "w_e18": we1.astype(np.float32).astype(f8),
        "bpk": np.ascontiguousarray(bias_pack),
        "xnTf": nT8,
    }
    in_maps = []
    for c in range(NCORES):
        sl = slice(c * SH, (c + 1) * SH)
        in_maps.append(
            {
                "xsT": np.ascontiguousarray(sT[:, sl]),
                "xnTl": np.ascontiguousarray(nT8[:, sl]),
                **shared,
            }
        )
    return in_maps


last_results = None


def _finish(results):
    total = 0.0
    for r in results:
        rowsum = r["out"].astype(np.float64)[:, :RT]  # [128, RT], col t = tile
        diag = r["out2"].astype(np.float64).reshape(-1)  # [SH] raw dots
        lse_flat = np.log(rowsum).T.reshape(-1)  # row j = t*128+p
        total += float((lse_flat - SCALE * diag).sum())
    return np.float32(total / B)


def kernel(**inputs) -> np.ndarray:
    from concourse.bass_utils import run_bass_kernel_spmd

    global last_results
    nc = _build()
    in_maps = _prep_in_maps(inputs)
    res = run_bass_kernel_spmd(nc, in_maps, core_ids=list(range(NCORES)))
    last_results = res
    return _finish(res.results)


# ---------------------------------------------------------------------------
# Pure-numpy golden model of the exact device algorithm (for test.py).
def golden(**inputs) -> np.ndarray:
    import ml_dtypes

    in_maps = _prep_in_maps(inputs)
    f32 = np.float32
    m0 = in_maps[0]
    bfd = ml_dtypes.bfloat16
    f8 = ml_dtypes.float8_e4m3

    def as32(x):
        return np.asarray(x).astype(f32)

    wpk = as32(m0["wpk"])
    we1 = as32(m0["we1b"])
    wgf = wpk[:, 0 : 3 * D]
    wp1 = wpk[:, 3 * D : 4 * D]
    wqf = wpk[:, 4 * D : 5 * D]
    bpk = m0["bpk"]
    we18 = as32(m0["w_e18"])

    # full fp8 layer-1 (same on every core)
    h1f = np.maximum(as32(m0["xnTf"]).T @ we18 + bpk[:, 0], 0.0)
    h1f8 = h1f.astype(f8).astype(f32)  # [B, D]

    def schraud(x):  # x = raw dot (unscaled); device V-group exp approx
        t = np.rint(x * EXP_A + EXP_B).astype(np.int16)
        return t.view(bfd).astype(f32)

    def b16(x):
        return x.astype(bfd).astype(f32)

    results = []
    for c in range(NCORES):
        m = in_maps[c]
        xsv = as32(m["xsT"]).T
        h1 = b16(np.maximum(xsv @ we1 + bpk[:, 0], 0.0))
        gi = h1 @ wgf
        r = 1.0 / (1.0 + np.exp(-(gi[:, :D] + bpk[:, 2])))
        zc = 1.0 / (1.0 + np.exp(gi[:, D : 2 * D] - bpk[:, 3]))
        n = np.tanh(gi[:, 2 * D :] + bpk[:, 4] + r * bpk[:, 5])
        ctx = b16(zc * n)
        h1p = b16(np.maximum(ctx @ wp1 + bpk[:, 6], 0.0))
        q8 = (h1p @ wqf + bpk[:, 7]).astype(f8).astype(f32)
        dots = q8 @ h1f8.T  # [SH, B] raw (unscaled)
        rowsum = np.zeros((SH,), f32)
        for g in range(NG):
            for t in range(RT):
                sub = dots[t * 128 : (t + 1) * 128, g * G : (g + 1) * G]
                if _is_v_group(g * RT + t):
                    es = schraud(sub)
                    ca = b16(es[:, : G // 2] + es[:, G // 2 :])
                    cb = b16(ca[:, : G // 4] + ca[:, G // 4 : G // 2])
                    s = cb.sum(axis=1)
                else:
                    s = np.exp(SCALE * sub.astype(np.float64)).astype(
                        np.float64
                    ).sum(axis=1)
                rowsum[t * 128 : (t + 1) * 128] += s.astype(f32)
        h1l = np.maximum(as32(m["xnTl"]).T @ we18 + bpk[:, 0], 0.0)
        h1l8 = h1l.astype(f8).astype(f32)
        prod = b16(q8 * h1l8)
        diag = prod.sum(axis=1)
        out = np.concatenate(
            [rowsum.reshape(RT, 128).T, np.zeros((128, RT), f32)], axis=1
        )
        results.append({"out": out, "out2": diag.reshape(1, SH)})
    return _finish(results)


# revision 4
# speedup vs baseline: 8421.5380x; 1.0958x over previous
"""Contrastive Predictive Coding loss kernel v2 for 8 Trainium2 NeuronCores.

Strategy (SPMD, batch row-sharded, per-core 1024 rows):
  - All activations transposed on chip ([D, batch], D on partitions); torch
    [in,out] weights used directly as lhsT. Host packs/folds weights.
  - Layer folds (host, f64): gi = h1s @ (we2@wih) + (be2@wih + bih + bhh)
    removes the encoder-l2 layer; q = h1p @ (wp2@we2^T) + bp2@we2^T removes
    the predictor-l2 layer (enc_b2's contribution is a per-row constant that
    cancels in LSE_i - logit_ii). GRU (1-z) computed as sigmoid(-x) directly.
  - fp8(e4m3) DoubleRow matmuls for the full-batch next_state layer-1 and the
    1024x8192 logits block; the local chain stays bf16.
  - Softmax stats on three engines: NS groups run fused exp+accum on ScalarE;
    the rest compute exp on VectorE via the bf16 Schraudolph bitcast
    (t = x*1846.64 + 16248.64 -> int16 -> bf16, bias-centered so
    E[approx/true] = 1), then reduce via two pairwise-add halvings on GpSimd
    (SBUF-only engine) and a final 512-wide reduce on VectorE.
  - diag(logits) via elementwise multiply + ones-vector matmul; result DMAed
    straight out of PSUM. log() of rowsums runs on host.
"""

import os
import sys

import numpy as np

for _p in ("/opt/trn_rl_repo", "/root/.axon_site/_ro/trn_rl_repo"):
    if os.path.isdir(_p) and _p not in sys.path:
        sys.path.append(_p)

D = 256
B = 8192
NCORES = 8
SH = B // NCORES          # 1024 rows per core
KP = D // 128             # 2 partition chunks of the contraction dim
NB = 512                  # matmul moving-operand block (one fp32 PSUM bank)
RT = SH // 128            # 8 row tiles of 128 pred rows
G = 2048                  # columns per exp/accum group (4 PSUM banks)
NG = B // G               # 4 groups per row tile
SCALE = 10.0              # 1 / temperature

EXP_A = SCALE * 128.0 / float(np.log(2.0))   # 1846.6407
EXP_B = 16256.0 - 7.364
NV = 14                   # of the 32 exp groups, how many go to V+Pool
NSPREAD = 28              # V-groups live in the first NSPREAD slots (clean tail)


def _is_v_group(idx):
    """Bresenham-spread NV V-groups among the first NSPREAD (g, t) slots."""
    if idx >= NSPREAD:
        return False
    return (idx * NV) // NSPREAD != ((idx + 1) * NV) // NSPREAD


_cache = {}


def _build():
    if "nc" in _cache:
        return _cache["nc"]

    import concourse.bacc as bacc
    import concourse.mybir as mybir
    import concourse.tile as tile

    dt = mybir.dt
    f32 = dt.float32
    bf16 = dt.bfloat16
    f8 = dt.float8e4
    i16 = dt.int16
    AF = mybir.ActivationFunctionType
    ALU = mybir.AluOpType
    AX = mybir.AxisListType
    DR = mybir.MatmulPerfMode.DoubleRow

    # packed bf16 weight columns inside wpk (per k-chunk of 128 rows)
    W_E1 = slice(0, D)                      # encoder l1
    W_G = slice(D, D + 3 * D)               # folded gru  (wgf = we2 @ wih)
    W_P1 = slice(4 * D, 5 * D)              # predictor l1
    W_Q = slice(5 * D, 6 * D)               # folded q    (wqf = wp2 @ we2^T)
    WPK_W = 6 * D

    nc = bacc.Bacc("TRN2", target_bir_lowering=False, num_devices=NCORES)

    xsT = nc.declare_dram_parameter("xsT", [D, SH], bf16, isOutput=False)
    xnTl = nc.declare_dram_parameter("xnTl", [D, SH], f8, isOutput=False)
    xnTf = nc.declare_dram_parameter("xnTf", [D, B], f8, isOutput=False)
    wpk_d = nc.declare_dram_parameter("wpk", [D, WPK_W], bf16, isOutput=False)
    w_e18 = nc.declare_dram_parameter("w_e18", [D, D], f8, isOutput=False)
    # bias columns: 0=enc_b1 2=gru_r 3=-(gru_z) 4=gru_n(bih) 5=gru_n(bhh)
    #               6=pred_b1 7=q (bp2 @ we2^T)
    bpk = nc.declare_dram_parameter("bpk", [D, 8], f32, isOutput=False)
    # out columns 0..RT-1 = exp-row-sums per row tile; out2 = raw diag dots
    out_d = nc.declare_dram_parameter("out", [128, 2 * RT], f32, isOutput=True)
    out2_d = nc.declare_dram_parameter("out2", [1, SH], f32, isOutput=True)

    with tile.TileContext(nc, num_cores=NCORES) as tc:
        with (
            tc.tile_pool(name="persist", bufs=1) as pp,
            tc.tile_pool(name="scratch", bufs=8) as sp,
        ):
            # ---- inputs, critical-path first: h1s needs xs + we1 (in wpk) ---
            # scalar queue: xs k=1, biases, wpk k=0, we18
            # sync queue:   xs k=0, wpk k=1, xnl8
            # gpsimd queue: xnf8 (SWDGE, chunked)
            # xnf8 chunk 0 heads the sync queue: layer-1 g0 needs it first
            # and the SWDGE (gpsimd) path has a slow cold start.
            xnf8 = pp.tile([128, KP, B], f8, name="xnf8")
            XCH = 2048
            for k in range(KP):
                nc.sync.dma_start(
                    out=xnf8[:, k, 0:XCH],
                    in_=xnTf[k * 128 : (k + 1) * 128, 0:XCH],
                )
            we18 = pp.tile([128, KP, D], f8, name="we18")
            for k in range(KP):  # scalar queue first: layer-1 starts on these
                nc.scalar.dma_start(
                    out=we18[:, k, :], in_=w_e18[k * 128 : (k + 1) * 128, :]
                )
            bp = []
            for k in range(KP):
                t = pp.tile([128, 8], f32, name=f"bp_{k}")
                nc.scalar.dma_start(out=t[:, :], in_=bpk[k * 128 : (k + 1) * 128, :])
                bp.append(t)
            xs = []
            for k in range(KP):  # sync queue
                t = pp.tile([128, SH], bf16, name=f"xs_{k}")
                nc.sync.dma_start(out=t[:, :], in_=xsT[k * 128 : (k + 1) * 128, :])
                xs.append(t)
            wall = []
            for k in range(KP):
                t = pp.tile([128, WPK_W], bf16, name=f"wall_{k}")
                q_eng = nc.scalar if k == 0 else nc.sync
                q_eng.dma_start(out=t[:, :], in_=wpk_d[k * 128 : (k + 1) * 128, :])
                wall.append(t)
            xnl8 = pp.tile([128, KP, SH], f8, name="xnl8")
            for k in range(KP):  # sync queue (needed post-chain only)
                nc.sync.dma_start(
                    out=xnl8[:, k, :], in_=xnTl[k * 128 : (k + 1) * 128, :]
                )
            # remaining next_state chunks ride behind the critical loads on
            # the serialized queues so they can't steal HBM bandwidth.
            for c0 in range(XCH, B, XCH):
                q_eng = (None, nc.gpsimd, nc.scalar, nc.sync)[c0 // XCH]
                for k in range(KP):
                    q_eng.dma_start(
                        out=xnf8[:, k, c0 : c0 + XCH],
                        in_=xnTf[k * 128 : (k + 1) * 128, c0 : c0 + XCH],
                    )
            ones = pp.tile([128, 1], bf16, name="ones")
            nc.vector.memset(ones[:, :], 1.0)

            def wslice(k, col, m):
                return wall[k][:, col.start + m * 128 : col.start + (m + 1) * 128]

            # persistent activations
            h1f8 = pp.tile([128, KP, B], f8, name="h1f8")
            qT8 = pp.tile([128, KP, SH], f8, name="qT8")
            dummy = pp.tile([128, G], f32, name="dummy")
            out_sb = pp.tile([128, 2 * RT], f32, name="out_sb")

            with tc.tile_pool(name="psx", bufs=2, space="PSUM") as psx:
                def ptile(name):
                    return psx.tile([128, G], f32, name=name, tag="px")

                def layer(dst, src, wcol, bias_col, act, tag):
                    """dst[m] (an AP) = act(w.T @ src + b) over SH cols; the
                    two 128-row output chunks epilogue on ScalarE (m=0) and
                    VectorE (m=1)."""
                    for m in range(KP):
                        ps = ptile(f"ps_{tag}_{m}")
                        for nb in range(SH // NB):
                            sl = slice(nb * NB, (nb + 1) * NB)
                            for k in range(KP):
                                nc.tensor.matmul(
                                    ps[:, sl],
                                    lhsT=wslice(k, wcol, m),
                                    rhs=src[k][:, sl],
                                    start=(k == 0),
                                    stop=(k == KP - 1),
                                )
                        bias = None if bias_col is None else bp[m][
                            :, bias_col : bias_col + 1
                        ]
                        if act == "relu" and m == 0:
                            nc.scalar.activation(
                                out=dst[m], in_=ps[:, :SH],
                                func=AF.Relu, bias=bias,
                            )
                        elif act == "relu":
                            nc.vector.tensor_scalar(
                                out=dst[m], in0=ps[:, :SH],
                                scalar1=bias, scalar2=0.0,
                                op0=ALU.add, op1=ALU.max,
                            )
                        elif act == "bias" and m == 0:
                            nc.scalar.activation(
                                out=dst[m], in_=ps[:, :SH],
                                func=AF.Identity, bias=bias,
                            )
                        else:
                            nc.vector.tensor_scalar(
                                out=dst[m], in0=ps[:, :SH],
                                scalar1=bias, scalar2=None, op0=ALU.add,
                            )

                def aps(tiles):
                    return [t[:, :] for t in tiles]

                # fp8 DoubleRow layer-1 over the FULL batch, one [128, G]
                # group at a time between chain stages.
                _l1_state = {"i": 0}

                def emit_l1(n=1):
                    for _ in range(n):
                        i = _l1_state["i"]
                        _l1_state["i"] += 1
                        if i >= KP * (B // G):
                            return
                        m, g0 = i % KP, (i // KP) * G
                        ps = ptile(f"ps_l1_{i}")
                        for s in range(G // NB):
                            sl = slice(s * NB, (s + 1) * NB)
                            gsl = slice(g0 + s * NB, g0 + (s + 1) * NB)
                            nc.tensor.matmul(
                                ps[:, sl],
                                lhsT=we18[:, :, m * 128 : (m + 1) * 128],
                                rhs=xnf8[:, :, gsl],
                                start=True, stop=True, perf_mode=DR,
                            )
                        dst = h1f8[:, m, g0 : g0 + G]
                        # alternate S/V; all run pre-logits so the exp act
                        # tables load once.
                        if i % 2 == 0:
                            nc.scalar.activation(
                                out=dst, in_=ps[:, :], func=AF.Relu,
                                bias=bp[m][:, 0:1],
                            )
                        else:
                            nc.vector.tensor_scalar(
                                out=dst, in0=ps[:, :],
                                scalar1=bp[m][:, 0:1], scalar2=0.0,
                                op0=ALU.add, op1=ALU.max,
                            )

                # layer-1 head start: PE crunches these while the chain's
                # weight DMAs land; continuous work also ramps the PE p-state.
                emit_l1(4)

                # ---- local chain: h1s -> GRU -> h1p -> q --------------------
                h1s = [sp.tile([128, SH], bf16, name=f"h1s_{k}", tag="scr") for k in range(KP)]
                layer(aps(h1s), xs, W_E1, 0, "relu", "h1s")
                emit_l1(2)

                def gate_psum(gate, m, tag):
                    ps = ptile(f"ps_{tag}_{m}")
                    col0 = W_G.start + gate * D + m * 128
                    for nb in range(SH // NB):
                        sl = slice(nb * NB, (nb + 1) * NB)
                        for k in range(KP):
                            nc.tensor.matmul(
                                ps[:, sl],
                                lhsT=wall[k][:, col0 : col0 + 128],
                                rhs=h1s[k][:, sl],
                                start=(k == 0),
                                stop=(k == KP - 1),
                            )
                    return ps

                rg = [sp.tile([128, SH], f32, name=f"rg_{m}", tag="scr") for m in range(KP)]
                zc = [sp.tile([128, SH], f32, name=f"zc_{m}", tag="scr") for m in range(KP)]
                ng = [sp.tile([128, SH], f32, name=f"ng_{m}", tag="scr") for m in range(KP)]
                ctx = [sp.tile([128, SH], bf16, name=f"ctx_{m}", tag="scr") for m in range(KP)]
                for m in range(KP):
                    ps_r = gate_psum(0, m, "gr")
                    nc.scalar.activation(
                        out=rg[m][:, :], in_=ps_r[:, :SH], func=AF.Sigmoid,
                        bias=bp[m][:, 2:3],
                    )
                emit_l1(2)
                for m in range(KP):
                    # zc = 1 - z = sigmoid(-(gi_z + b_z)); bias col3 is -b_z
                    ps_z = gate_psum(1, m, "gz")
                    nc.scalar.activation(
                        out=zc[m][:, :], in_=ps_z[:, :SH], func=AF.Sigmoid,
                        scale=-1.0, bias=bp[m][:, 3:4],
                    )
                for m in range(KP):
                    ps_n = gate_psum(2, m, "gn")
                    tmp = sp.tile([128, SH], f32, name=f"tmp_{m}", tag="scr")
                    # tmp = r * bhh_n + gi_n   (one fused DVE op)
                    nc.vector.scalar_tensor_tensor(
                        out=tmp[:, :], in0=rg[m][:, :], scalar=bp[m][:, 5:6],
                        in1=ps_n[:, :SH], op0=ALU.mult, op1=ALU.add,
                    )
                    nc.scalar.activation(
                        out=ng[m][:, :], in_=tmp[:, :], func=AF.Tanh,
                        bias=bp[m][:, 4:5],
                    )
                    nc.vector.tensor_tensor(
                        out=ctx[m][:, :], in0=zc[m][:, :], in1=ng[m][:, :],
                        op=ALU.mult,
                    )

                h1p = [sp.tile([128, SH], bf16, name=f"h1p_{k}", tag="scr") for k in range(KP)]
                layer(aps(h1p), ctx, W_P1, 6, "relu", "h1p")
                # q = h1p @ wqf + bq (identity epilogue, fp8 out)
                layer([qT8[:, 0, :], qT8[:, 1, :]], h1p, W_Q, 7, "bias", "q")

                # local next-state relu layer (for the diagonal): fp8 DR
                h1nl8 = pp.tile([128, KP, SH], f8, name="h1nl8")
                for m in range(KP):
                    ps = ptile(f"ps_h1nl_{m}")
                    for s in range(SH // NB):
                        sl = slice(s * NB, (s + 1) * NB)
                        nc.tensor.matmul(
                            ps[:, sl],
                            lhsT=we18[:, :, m * 128 : (m + 1) * 128],
                            rhs=xnl8[:, :, sl],
                            start=True, stop=True, perf_mode=DR,
                        )
                    nc.vector.tensor_scalar(
                        out=h1nl8[:, m, :], in0=ps[:, :SH],
                        scalar1=bp[m][:, 0:1], scalar2=0.0,
                        op0=ALU.add, op1=ALU.max,
                    )
                prod = [sp.tile([128, SH], bf16, name=f"prod_{k}", tag="scr") for k in range(KP)]
                for k in range(KP):
                    nc.vector.tensor_tensor(
                        out=prod[k][:, :], in0=qT8[:, k, :], in1=h1nl8[:, k, :],
                        op=ALU.mult,
                    )

                # ---- logits: fp8 DR matmuls; exp on S or V+Pool -------------
                sums = pp.tile([128, RT * NG], f32, name="sums")
                for g in range(NG):
                    for t in range(RT):
                        pl = ptile("pl")
                        for s in range(G // NB):
                            c0 = g * G + s * NB
                            sl = slice(s * NB, (s + 1) * NB)
                            nc.tensor.matmul(
                                pl[:, sl],
                                lhsT=qT8[:, :, t * 128 : (t + 1) * 128],
                                rhs=h1f8[:, :, c0 : c0 + NB],
                                start=True, stop=True, perf_mode=DR,
                            )
                        col = t * NG + g
                        if _is_v_group(g * RT + t):
                            ex = sp.tile([128, G], i16, name="ex", tag="ex")
                            nc.vector.tensor_scalar(
                                out=ex[:, :], in0=pl[:, :],
                                scalar1=EXP_A, scalar2=EXP_B,
                                op0=ALU.mult, op1=ALU.add,
                            )
                            ca = sp.tile([128, G // 2], bf16, name="ca", tag="ca")
                            nc.gpsimd.tensor_tensor(
                                out=ca[:, :],
                                in0=ex[:, 0 : G // 2].bitcast(bf16),
                                in1=ex[:, G // 2 : G].bitcast(bf16),
                                op=ALU.add,
                            )
                            cb = sp.tile([128, G // 4], bf16, name="cb", tag="cb")
                            nc.gpsimd.tensor_tensor(
                                out=cb[:, :],
                                in0=ca[:, 0 : G // 4],
                                in1=ca[:, G // 4 : G // 2],
                                op=ALU.add,
                            )
                            nc.vector.tensor_reduce(
                                out=sums[:, col : col + 1],
                                in_=cb[:, :], axis=AX.X, op=ALU.add,
                            )
                        else:
                            nc.scalar.activation(
                                out=dummy[:, :], in_=pl[:, :], func=AF.Exp,
                                scale=SCALE,
                                accum_out=sums[:, col : col + 1],
                            )
                # ---- diag: rowsum(q8 * h1nl8) via ones-vector matmul --------
                dps = ptile("dps")
                for s in range(SH // NB):
                    sl = slice(s * NB, (s + 1) * NB)
                    for k in range(KP):
                        nc.tensor.matmul(
                            dps[:1, sl],
                            lhsT=ones[:, :],
                            rhs=prod[k][:, sl],
                            start=(k == 0),
                            stop=(k == KP - 1),
                        )
                d2 = pp.tile([1, SH], f32, name="d2")
                nc.scalar.copy(d2[:, :], dps[:1, :SH])
                nc.sync.dma_start(out=out2_d[:, :], in_=d2[:, :])
                for t in range(RT):
                    nc.vector.reduce_sum(
                        out=out_sb[:, t : t + 1],
                        in_=sums[:, t * NG : (t + 1) * NG], axis=AX.X,
                    )

            nc.sync.dma_start(out=out_d[:, :], in_=out_sb[:, :])

    if not nc.is_finalized():
        nc.finalize()
    _cache["nc"] = nc
    return nc


def _prep_in_maps(inputs):
    import ml_dtypes

    bf = ml_dtypes.bfloat16
    f8 = ml_dtypes.float8_e4m3
    f = lambda x: np.asarray(x).astype(np.float64)
    state = f(inputs["state"])
    next_state = f(inputs["next_state"])
    we1 = f(inputs["enc_w1"])
    we2 = f(inputs["enc_w2"])
    wih = f(inputs["gru_wih"])
    wp1 = f(inputs["pred_w1"])
    wp2 = f(inputs["pred_w2"])
    be1 = f(inputs["enc_b1"])
    be2 = f(inputs["enc_b2"])
    bih = f(inputs["gru_bih"])
    bhh = f(inputs["gru_bhh"])
    bp1 = f(inputs["pred_b1"])
    bp2 = f(inputs["pred_b2"])

    wgf = we2 @ wih                    # [D, 3D] folded gru weights
    bgf = be2 @ wih + bih              # [3D]    folded gi bias (+bih)
    wqf = wp2 @ we2.T                  # [D, D]  folded q weights
    bqf = bp2 @ we2.T                  # [D]     folded q bias

    bias_pack = np.stack(
        [
            be1,
            np.zeros(D),
            bgf[:D] + bhh[:D],                    # r-gate bias
            -(bgf[D : 2 * D] + bhh[D : 2 * D]),   # NEGATED z-gate bias
            bgf[2 * D :],                         # n-gate (bih part)
            bhh[2 * D :],                         # n-gate (r * bhh part)
            bp1,
            bqf,
        ],
        axis=1,
    ).astype(np.float32)

    wpk = np.concatenate([we1, wgf, wp1, wqf], axis=1)  # [D, 6D]
    nT8 = np.ascontiguousarray(next_state.T.astype(np.float32)).astype(f8)
    sT = np.ascontiguousarray(state.T.astype(np.float32)).astype(bf)
    shared = {
        "wpk": np.ascontiguousarray(wpk.astype(np.float32)).astype(bf),
        "w_e18": we1.astype(np.float32).astype(f8),
        "bpk": np.ascontiguousarray(bias_pack),
        "xnTf": nT8,
    }
    in_maps = []
    for c in range(NCORES):
        sl = slice(c * SH, (c + 1) * SH)
        in_maps.append(
            {
                "xsT": np.ascontiguousarray(sT[:, sl]),
                "xnTl": np.ascontiguousarray(nT8[:, sl]),
                **shared,
            }
        )
    return in_maps


last_results = None


def _finish(results):
    total = 0.0
    for r in results:
        rowsum = r["out"].astype(np.float64)[:, :RT]  # [128, RT], col t = tile
        diag = r["out2"].astype(np.float64).reshape(-1)  # [SH] raw dots
        lse_flat = np.log(rowsum).T.reshape(-1)  # row j = t*128+p
        total += float((lse_flat - SCALE * diag).sum())
    return np.float32(total / B)


def kernel(**inputs) -> np.ndarray:
    from concourse.bass_utils import run_bass_kernel_spmd

    global last_results
    nc = _build()
    in_maps = _prep_in_maps(inputs)
    res = run_bass_kernel_spmd(nc, in_maps, core_ids=list(range(NCORES)))
    last_results = res
    return _finish(res.results)


# ---------------------------------------------------------------------------
# Pure-numpy golden model of the exact device algorithm (for test.py).
def golden(**inputs) -> np.ndarray:
    import ml_dtypes

    in_maps = _prep_in_maps(inputs)
    f32 = np.float32
    m0 = in_maps[0]
    bfd = ml_dtypes.bfloat16
    f8 = ml_dtypes.float8_e4m3

    def as32(x):
        return np.asarray(x).astype(f32)

    wpk = as32(m0["wpk"])
    we1 = wpk[:, :D]
    wgf = wpk[:, D : 4 * D]
    wp1 = wpk[:, 4 * D : 5 * D]
    wqf = wpk[:, 5 * D : 6 * D]
    bpk = m0["bpk"]
    we18 = as32(m0["w_e18"])

    # full fp8 layer-1 (same on every core)
    h1f = np.maximum(as32(m0["xnTf"]).T @ we18 + bpk[:, 0], 0.0)
    h1f8 = h1f.astype(f8).astype(f32)  # [B, D]

    def schraud(x):  # x = raw dot (unscaled); device V-group exp approx
        t = np.rint(x * EXP_A + EXP_B).astype(np.int16)
        return t.view(bfd).astype(f32)

    def b16(x):
        return x.astype(bfd).astype(f32)

    results = []
    for c in range(NCORES):
        m = in_maps[c]
        xsv = as32(m["xsT"]).T
        h1 = b16(np.maximum(xsv @ we1 + bpk[:, 0], 0.0))
        gi = h1 @ wgf
        r = 1.0 / (1.0 + np.exp(-(gi[:, :D] + bpk[:, 2])))
        zc = 1.0 / (1.0 + np.exp(gi[:, D : 2 * D] - bpk[:, 3]))
        n = np.tanh(gi[:, 2 * D :] + bpk[:, 4] + r * bpk[:, 5])
        ctx = b16(zc * n)
        h1p = b16(np.maximum(ctx @ wp1 + bpk[:, 6], 0.0))
        q8 = (h1p @ wqf + bpk[:, 7]).astype(f8).astype(f32)
        dots = q8 @ h1f8.T  # [SH, B] raw (unscaled)
        rowsum = np.zeros((SH,), f32)
        for g in range(NG):
            for t in range(RT):
                sub = dots[t * 128 : (t + 1) * 128, g * G : (g + 1) * G]
                if _is_v_group(g * RT + t):
                    es = schraud(sub)
                    ca = b16(es[:, : G // 2] + es[:, G // 2 :])
                    cb = b16(ca[:, : G // 4] + ca[:, G // 4 : G // 2])
                    s = cb.sum(axis=1)
                else:
                    s = np.exp(SCALE * sub.astype(np.float64)).astype(
                        np.float64
                    ).sum(axis=1)
                rowsum[t * 128 : (t + 1) * 128] += s.astype(f32)
        h1l = np.maximum(as32(m["xnTl"]).T @ we18 + bpk[:, 0], 0.0)
        h1l8 = h1l.astype(f8).astype(f32)
        prod = b16(q8 * h1l8)
        diag = prod.sum(axis=1)
        out = np.concatenate(
            [rowsum.reshape(RT, 128).T, np.zeros((128, RT), f32)], axis=1
        )
        results.append({"out": out, "out2": diag.reshape(1, SH)})
    return _finish(results)
